# revision 8
# baseline (speedup 1.0000x reference)
"""Multi-Query Attention kernel for 8x TRN2 NeuronCores (Bass/Tile).

Problem: x[B=2, L=2048, D=2048], Wq[2048,2048], Wk/Wv[128,2048] (MQA: one
shared K/V head), 16 query heads of dim 128.

Sharding: core c in [0,8): batch b = c//4, head-group g = c%4 (4 heads,
i.e. q-channels [512g, 512g+512)). K/V replicated per core.

V3 design (all-bf16 matmul path, phase-fused, stall-free pass boundaries):
  - x resident in SBUF, DMA'd in [128,512] pieces aligned with the l-tiles
    the projections consume, so the PE never waits long on one queue
  - all matmuls bf16 (same PE rate as f32r, but FWL + LDWEIGHTS pull-ahead
    hide weight loads). fp8/DoubleRow rejected: ~2.4% RMS quantization
    noise propagates undamped through dots -> breaches the 2e-2 gate.
  - phase 1: K/V projections (+ PE transposes of V to natural layout);
    Q projection for q-block 0 folded into the later l-tiles once its
    weights have arrived
  - phase 2: per q-block: attention passes with the NEXT q-block's Q
    projection interleaved one 2-matmul step per iteration (PE stays busy
    while ACT chews the 1.15us-per-tile exp stream)
  - softmax denominator: 4 interleaved partial-sum chains over the exp'd
    tiles (3 on DVE, 1 on GpSimd) - independent chains hide the DVE
    read-write bubble; one ones-matmul per head reduces over key
    partitions into a dedicated 1-bank PSUM slot
  - pass finals are DEFERRED: the denominator matmul + reciprocal +
    normalize of pass N are emitted in the middle of pass N+1, and all
    PSUM tiles (AV accumulators, r) are evacuated to SBUF immediately, so
    no PSUM rotation ever waits on the multi-microsecond DVE reciprocal.
    (V2 stalled ~6us at every q-block boundary on exactly this, which also
    re-throttled the PE clock via HAM.)
  - head 0 of each pass normalizes via DVE reciprocal+mul, head 1 via a
    GpSimd divide, balancing the two engines
  - PSUM: Qproj 1 + scores 2x2 + AV 2 + r 1 = 8 banks exactly
"""

from contextlib import ExitStack

import numpy as np
import ml_dtypes

import concourse.bass as bass
import concourse.tile as tile
from concourse import bacc, masks, mybir
from concourse.bass_utils import run_bass_kernel_spmd

F32 = mybir.dt.float32
BF16 = mybir.dt.bfloat16
AF = mybir.ActivationFunctionType
ADD = mybir.AluOpType.add
DIVIDE = mybir.AluOpType.divide

B = 2
L = 2048
D = 2048  # d_model (contraction dim of projections)
HD = 128  # head dim
NH = 4  # heads per core
QC = NH * HD  # q-channels per core = 512
DC = D // 128  # d-model chunks of 128 = 16
NLT = 4  # l tiles of 512 (projection)
LKT = L // 128  # lk blocks of 128 = 16
NLQ = 4  # lq blocks of 512 (attention)
N_CORES = 8
SCALE = 1.0 / float(np.sqrt(HD))

GP_CHAINS = 2  # how many of the 4 partial-sum chains run on GpSimd


def build_kernel(ctx: ExitStack, tc: tile.TileContext, xT, wqT, wkT, wvT, bq, bk, bv, outT):
    nc = tc.nc

    persist = ctx.enter_context(tc.tile_pool(name="persist", bufs=1))
    x_sb = [persist.tile([128, L], BF16, tag=f"x{k}", name=f"x{k}") for k in range(DC)]
    qT = [persist.tile([128, L], BF16, tag=f"qT{h}", name=f"qT{h}") for h in range(NH)]  # [d, l]
    kT = persist.tile([128, L], BF16, tag="kT", name="kT")  # [d, l]
    vN = persist.tile([128, L], BF16, tag="vN", name="vN")  # block j: [:, 128j:+128] = V[128j:+128, :]
    wq_ch = [persist.tile([128, QC], BF16, tag=f"wq{k}", name=f"wq{k}") for k in range(DC)]
    wk_ch = [persist.tile([128, HD], BF16, tag=f"wk{k}", name=f"wk{k}") for k in range(DC)]
    wv_ch = [persist.tile([128, HD], BF16, tag=f"wv{k}", name=f"wv{k}") for k in range(DC)]
    ones = persist.tile([128, 128], BF16, tag="ones", name="ones")
    ident = persist.tile([128, 128], BF16, tag="ident", name="ident")
    bq_sb = persist.tile([128, NH], F32, tag="bq", name="bq")
    bqr_sb = persist.tile([128, NH * 512], F32, tag="bqr", name="bqr")
    bk_sb = persist.tile([128, 1], F32, tag="bk", name="bk")
    bv_sb = persist.tile([128, 1], F32, tag="bv", name="bv")

    nc.vector.memset(ones[:], 1.0)
    nc.vector.memset(bqr_sb[:], 0.0)
    masks.make_identity(nc, ident[:])
    nc.sync.dma_start(out=bq_sb[:], in_=bq)
    nc.sync.dma_start(out=bk_sb[:], in_=bk)
    nc.sync.dma_start(out=bv_sb[:], in_=bv)
    # per-head Q bias replicated along l (for DVE-side evacuation adds):
    # out = 0*0 + bias via ACT, no DMA needed
    for h in range(NH):
        nc.scalar.activation(
            bqr_sb[:, h * 512:(h + 1) * 512], bqr_sb[:, h * 512:(h + 1) * 512],
            AF.Identity, scale=0.0, bias=bq_sb[:, h:h + 1],
        )

    # Input DMAs in 4 waves of [128,512] x-pieces so arrival tracks the
    # l-tile order phase 1 consumes; weights ride along (wq before wave 1
    # because the Q0 projection runs in l-tiles 2 and 3). Transfers
    # alternate between the SP and Activation HW DGE queues — a single
    # queue caps out around 200 GB/s and paced all of phase 1 in V3.
    dq = [nc.sync, nc.scalar]
    for k in range(DC):
        rs = slice(k * 128, (k + 1) * 128)
        dq[k % 2].dma_start(out=wk_ch[k][:], in_=wkT[rs, :])
        dq[(k + 1) % 2].dma_start(out=wv_ch[k][:], in_=wvT[rs, :])
        dq[k % 2].dma_start(out=x_sb[k][:, 0:512], in_=xT[rs, 0:512])
    for k in range(DC):
        rs = slice(k * 128, (k + 1) * 128)
        dq[k % 2].dma_start(out=wq_ch[k][:], in_=wqT[rs, :])
        dq[(k + 1) % 2].dma_start(out=x_sb[k][:, 512:1024], in_=xT[rs, 512:1024])
    for k in range(DC):
        rs = slice(k * 128, (k + 1) * 128)
        dq[k % 2].dma_start(out=x_sb[k][:, 1024:1536], in_=xT[rs, 1024:1536])
        dq[(k + 1) % 2].dma_start(out=x_sb[k][:, 1536:2048], in_=xT[rs, 1536:2048])

    # ---------------- Phase 1: K/V projections (all L) + Q for lq=0 ----------
    with (
        tc.tile_pool(name="pjkv", bufs=2, space="PSUM") as pjkv,
        tc.tile_pool(name="q0p", bufs=1, space="PSUM") as q0p,
        tc.tile_pool(name="tp", bufs=2, space="PSUM") as tpp,
        tc.tile_pool(name="vt", bufs=2) as vtp,
    ):
        for lt in range(NLT):
            ls = slice(lt * 512, (lt + 1) * 512)
            psk = pjkv.tile([128, 512], F32, tag="a", name=f"psk{lt}")
            psv = pjkv.tile([128, 512], F32, tag="b", name=f"psv{lt}")
            # Q0 half-pass (2 heads) folded into l-tiles 2 and 3
            qh = None
            if lt >= 2:
                h0, h1 = 2 * (lt - 2), 2 * (lt - 2) + 1
                pa = q0p.tile([128, 512], F32, tag="qa", name=f"q0a{lt}")
                pb = q0p.tile([128, 512], F32, tag="qb", name=f"q0b{lt}")
                qh = (h0, h1, pa, pb)
            for k in range(DC):
                st, sp = k == 0, k == DC - 1
                nc.tensor.matmul(psk[:], lhsT=wk_ch[k][:], rhs=x_sb[k][:, ls], start=st, stop=sp)
                nc.tensor.matmul(psv[:], lhsT=wv_ch[k][:], rhs=x_sb[k][:, ls], start=st, stop=sp)
                if qh is not None:
                    h0, h1, pa, pb = qh
                    nc.tensor.matmul(pa[:], lhsT=wq_ch[k][:, h0 * 128:(h0 + 1) * 128], rhs=x_sb[k][:, 0:512], start=st, stop=sp)
                    nc.tensor.matmul(pb[:], lhsT=wq_ch[k][:, h1 * 128:(h1 + 1) * 128], rhs=x_sb[k][:, 0:512], start=st, stop=sp)
            nc.scalar.activation(kT[:, ls], psk[:], AF.Identity, bias=bk_sb[:, 0:1])
            vt = vtp.tile([128, 512], BF16, tag="vt", name=f"vt{lt}")
            nc.scalar.activation(vt[:], psv[:], AF.Identity, bias=bv_sb[:, 0:1])
            if qh is not None:
                h0, h1, pa, pb = qh
                nc.scalar.activation(qT[h0][:, 0:512], pa[:], AF.Identity, bias=bq_sb[:, h0:h0 + 1])
                nc.scalar.activation(qT[h1][:, 0:512], pb[:], AF.Identity, bias=bq_sb[:, h1:h1 + 1])
            # transpose this l-tile of V to natural layout right away
            for jj in range(4):
                j = lt * 4 + jj
                pt = tpp.tile([128, 128], BF16, tag="tp", name=f"tp{j}")
                nc.tensor.transpose(pt[:], vt[:, jj * 128:(jj + 1) * 128], ident[:])
                nc.scalar.activation(vN[:, j * 128:(j + 1) * 128], pt[:], AF.Identity)

    # ---------------- Phase 2: attention + interleaved Q-proj ----------------
    with (
        tc.tile_pool(name="pj2", bufs=1, space="PSUM") as pj2,  # 1 bank
        tc.tile_pool(name="ssp", bufs=2, space="PSUM") as ssp,  # 4 banks
        tc.tile_pool(name="avp", bufs=1, space="PSUM") as avp,  # 2 banks
        tc.tile_pool(name="rrp", bufs=1, space="PSUM") as rrp,  # 1 bank
        tc.tile_pool(name="att", bufs=10) as attp,
        tc.tile_pool(name="acc", bufs=2) as accp,
        tc.tile_pool(name="fin", bufs=2) as finp,
    ):
        def emit_av(p):
            at, psA, ks, st, sp = p
            for j in range(2):
                nc.tensor.matmul(
                    psA[j][:], lhsT=vN[:, ks], rhs=at[:, j * 512:(j + 1) * 512],
                    start=st, stop=sp,
                )

        def make_qproj(lq):
            """32 steps x (2 matmuls of one k-chunk-pair) of the Q projection
            for q-block lq+1, as 4 single-head quarter passes (1 PSUM bank)."""
            if lq >= NLQ - 1:
                while True:
                    yield
            nls = slice((lq + 1) * 512, (lq + 2) * 512)
            for h in range(NH):
                pq = pj2.tile([128, 512], F32, tag="q", name=f"pq{lq}{h}")
                hs = slice(h * 128, (h + 1) * 128)
                for s in range(8):
                    for k in (2 * s, 2 * s + 1):
                        nc.tensor.matmul(pq[:], lhsT=wq_ch[k][:, hs], rhs=x_sb[k][:, nls], start=k == 0, stop=k == DC - 1)
                    if s == 7:
                        # evac with bias: per-partition bias pre-replicated
                        # along l on the host (bqr), so DVE can do it
                        nc.vector.tensor_add(qT[h][:, nls], pq[:], bqr_sb[:, h * 512:(h + 1) * 512])
                    yield
            while True:
                yield

        def make_finalize(j, h, qs, fin, otu):
            def fi():
                rr = rrp.tile([128, 512], F32, tag="rr", name=f"rr{h}{qs.start}")
                nc.tensor.matmul(rr[:], lhsT=ones[:], rhs=fin[:, j * 512:(j + 1) * 512], start=True, stop=True)
                rrs = finp.tile([128, 512], F32, tag="rrs", name=f"rrs{h}{qs.start}")
                nc.vector.tensor_copy(rrs[:], rr[:])
                ot = finp.tile([128, 512], F32, tag="ot", name=f"o{h}{qs.start}")
                rinv = finp.tile([128, 512], F32, tag="rinv", name=f"ri{h}{qs.start}")
                nc.vector.reciprocal(rinv[:], rrs[:])
                # normalize multiply on GpSimd: all operands SBUF, and it
                # keeps the multi-microsecond finals chain off DVE's queue
                nc.gpsimd.tensor_mul(ot[:], otu[:], rinv[:])
                nc.sync.dma_start(out=outT[h * 128:(h + 1) * 128, qs], in_=ot[:])
            return fi

        deferred = []
        for lq in range(NLQ):
            qs = slice(lq * 512, (lq + 1) * 512)
            qsteps = make_qproj(lq)
            for hp in range(2):
                psA = [avp.tile([128, 512], F32, tag=f"av{j}", name=f"av{lq}{hp}{j}") for j in range(2)]
                pend = []
                chains = [None] * 4
                for lk in range(LKT):
                    ks = slice(lk * 128, (lk + 1) * 128)
                    ss = ssp.tile([128, 1024], F32, tag="ss", name=f"ss{lq}{hp}{lk}")
                    for j in range(2):
                        nc.tensor.matmul(
                            ss[:, j * 512:(j + 1) * 512],
                            lhsT=kT[:, ks], rhs=qT[2 * hp + j][:, qs],
                            start=True, stop=True,
                        )
                    at = attp.tile([128, 1024], BF16, tag="at", name=f"at{lq}{hp}{lk}")
                    nc.scalar.activation(at[:], ss[:], AF.Exp, scale=SCALE)
                    # 4 interleaved partial-sum chains (independent -> no
                    # per-op read-write bubble); chain c < GP_CHAINS on GpSimd
                    c = lk % 4
                    if lk < 4:
                        chains[c] = at
                    else:
                        a_new = accp.tile([128, 1024], BF16, tag=f"ch{c}", name=f"ch{lq}{hp}{lk}")
                        eng = nc.gpsimd if c < GP_CHAINS else nc.vector
                        eng.tensor_add(a_new[:], chains[c][:], at[:])
                        chains[c] = a_new
                    # software-pipelined AV (consume an older chunk's exp)
                    pend.append((at, psA, ks, lk == 0, lk == LKT - 1))
                    if len(pend) > 4:
                        emit_av(pend.pop(0))
                    # previous pass's finals, mid-pass so nothing waits
                    if lk == 2 and deferred:
                        deferred.pop(0)()
                    if lk == 5 and deferred:
                        deferred.pop(0)()
                    next(qsteps)
                while pend:
                    emit_av(pend.pop(0))
                # combine chains (DVE), evacuate AV PSUM to SBUF right away
                t1 = accp.tile([128, 1024], BF16, tag="cA", name=f"cA{lq}{hp}")
                nc.vector.tensor_add(t1[:], chains[0][:], chains[1][:])
                t2 = accp.tile([128, 1024], BF16, tag="cB", name=f"cB{lq}{hp}")
                nc.vector.tensor_add(t2[:], chains[2][:], chains[3][:])
                fin = accp.tile([128, 1024], BF16, tag="fin", name=f"fin{lq}{hp}")
                nc.vector.tensor_add(fin[:], t1[:], t2[:])
                for j in range(2):
                    otu = finp.tile([128, 512], F32, tag=f"otu{j}", name=f"otu{lq}{hp}{j}")
                    nc.vector.tensor_copy(otu[:], psA[j][:])
                    deferred.append(make_finalize(j, 2 * hp + j, qs, fin, otu))
        while deferred:
            deferred.pop(0)()


_NC_CACHE = None


def build_nc():
    global _NC_CACHE
    if _NC_CACHE is not None:
        return _NC_CACHE
    nc = bacc.Bacc("TRN2", target_bir_lowering=False, debug=False)
    xT = nc.dram_tensor("xT", [D, L], BF16, kind="ExternalInput").ap()
    wqT = nc.dram_tensor("wqT", [D, QC], BF16, kind="ExternalInput").ap()
    wkT = nc.dram_tensor("wkT", [D, HD], BF16, kind="ExternalInput").ap()
    wvT = nc.dram_tensor("wvT", [D, HD], BF16, kind="ExternalInput").ap()
    bq = nc.dram_tensor("bq", [128, NH], F32, kind="ExternalInput").ap()
    bk = nc.dram_tensor("bk", [128, 1], F32, kind="ExternalInput").ap()
    bv = nc.dram_tensor("bv", [128, 1], F32, kind="ExternalInput").ap()
    outT = nc.dram_tensor("outT", [QC, L], F32, kind="ExternalOutput").ap()
    with tile.TileContext(nc) as tc, ExitStack() as ctx:
        build_kernel(ctx, tc, xT, wqT, wkT, wvT, bq, bk, bv, outT)
    nc.compile()
    _NC_CACHE = nc
    return nc


def make_in_maps(x, Wq_w, Wq_b, Wk_w, Wk_b, Wv_w, Wv_b):
    """Host-side sharding/relayout. Returns one input map per core."""
    bf16 = ml_dtypes.bfloat16
    x = np.asarray(x, dtype=np.float32)
    Wq_w = np.asarray(Wq_w, dtype=np.float32)
    Wq_b = np.asarray(Wq_b, dtype=np.float32)
    Wk_w = np.asarray(Wk_w, dtype=np.float32)
    Wk_b = np.asarray(Wk_b, dtype=np.float32)
    Wv_w = np.asarray(Wv_w, dtype=np.float32)
    Wv_b = np.asarray(Wv_b, dtype=np.float32)

    xTs = [np.ascontiguousarray(x[b].T).astype(bf16) for b in range(B)]
    wkT = np.ascontiguousarray(Wk_w.T).astype(bf16)
    wvT = np.ascontiguousarray(Wv_w.T).astype(bf16)
    bk = np.ascontiguousarray(Wk_b.reshape(128, 1))
    bv = np.ascontiguousarray(Wv_b.reshape(128, 1))
    in_maps = []
    for c in range(N_CORES):
        b, g = divmod(c, B * 2)  # b = c // 4, g = c % 4
        wqT_g = np.ascontiguousarray(Wq_w[g * QC:(g + 1) * QC, :].T).astype(bf16)
        bq_g = np.ascontiguousarray(Wq_b[g * QC:(g + 1) * QC].reshape(NH, 128).T)
        in_maps.append(
            {
                "xT": xTs[b],
                "wqT": wqT_g,
                "wkT": wkT,
                "wvT": wvT,
                "bq": bq_g,
                "bk": bk,
                "bv": bv,
            }
        )
    return in_maps


def assemble_output(results):
    out = np.empty((B, L, D), dtype=np.float32)
    for c in range(N_CORES):
        b, g = divmod(c, B * 2)
        out[b, :, g * QC:(g + 1) * QC] = results[c]["outT"].T
    return out


def kernel(**inputs) -> np.ndarray:
    nc = build_nc()
    in_maps = make_in_maps(**inputs)
    res = run_bass_kernel_spmd(nc, in_maps, core_ids=list(range(N_CORES)))
    return assemble_output(res.results)


# revision 9
# speedup vs baseline: 1.1117x; 1.1117x over previous
"""Multi-Query Attention kernel for 8x TRN2 NeuronCores (Bass/Tile).

Problem: x[B=2, L=2048, D=2048], Wq[2048,2048], Wk/Wv[128,2048] (MQA: one
shared K/V head), 16 query heads of dim 128.

Sharding: core c in [0,8): batch b = c//4, head-group g = c%4 (4 heads,
i.e. q-channels [512g, 512g+512)). K/V replicated per core.

V5 design (all-bf16 matmul path, phase-fused, stall-free pass boundaries):
  - x resident in SBUF, DMA'd as whole [128,2048] chunks (4KB partition
    lines; small lines ran at ~half DMA rate and paced all of phase 1),
    transfers alternating between the SP and Activation HW DGE queues;
    Wq/Wk/Wv are concatenated host-side into one [D,768] tensor so weight
    chunks also move with 1.5KB lines
  - all matmuls bf16 (same PE rate as f32r, but FWL + LDWEIGHTS pull-ahead
    hide weight loads). fp8/DoubleRow rejected: ~2.4% RMS quantization
    noise propagates undamped through dots -> breaches the 2e-2 gate.
  - phase 1: K/V projections (+ PE transposes of V to natural layout);
    Q projection for q-block 0 folded into l-tiles 2/3
  - phase 2: per q-block: attention passes with the NEXT q-block's Q
    projection interleaved one 2-matmul step per iteration (PE stays busy
    while ACT chews the 1.15us-per-tile exp stream)
  - softmax denominator: 4 interleaved partial-sum chains over the exp'd
    tiles (2 on GpSimd, 2 on DVE; fully serial chains pay a read-write
    bubble per op, and GpSimd contends with DVE for the shared SBUF port,
    so the load is split), pairwise-combined on DVE; the cross-partition
    reduction is two accumulating ones-matmuls per head into a dedicated
    1-bank PSUM slot
  - pass finals are DEFERRED into the middle of the NEXT pass and fully
    decoupled from PSUM (AV accumulators and r are evacuated to SBUF
    immediately), so no PSUM rotation and no PE instruction ever waits on
    the multi-microsecond reciprocal; head 0 inverts on DVE, head 1 on
    ACT via exp(-ln(r)) (both functions live in one table set)
  - PSUM: Qproj 1 + scores 2x2 + AV 2 + r 1 = 8 banks exactly
"""

from contextlib import ExitStack

import numpy as np
import ml_dtypes

import concourse.bass as bass
import concourse.tile as tile
from concourse import bacc, masks, mybir
from concourse.bass_utils import run_bass_kernel_spmd

F32 = mybir.dt.float32
BF16 = mybir.dt.bfloat16
AF = mybir.ActivationFunctionType

B = 2
L = 2048
D = 2048  # d_model (contraction dim of projections)
HD = 128  # head dim
NH = 4  # heads per core
QC = NH * HD  # q-channels per core = 512
DC = D // 128  # d-model chunks of 128 = 16
NLT = 4  # l tiles of 512 (projection)
LKT = L // 128  # lk blocks of 128 = 16
NLQ = 4  # lq blocks of 512 (attention)
WQK = QC + 2 * HD  # combined weight row: 512 wq | 128 wk | 128 wv
N_CORES = 8
SCALE = 1.0 / float(np.sqrt(HD))

GP_CHAINS = 2  # how many of the 4 partial-sum chains run on GpSimd


def build_kernel(ctx: ExitStack, tc: tile.TileContext, xT, wall, bq, bk, bv, outT):
    nc = tc.nc

    persist = ctx.enter_context(tc.tile_pool(name="persist", bufs=1))
    x_sb = [persist.tile([128, L], BF16, tag=f"x{k}", name=f"x{k}") for k in range(DC)]
    qT = [persist.tile([128, L], BF16, tag=f"qT{h}", name=f"qT{h}") for h in range(NH)]  # [d, l]
    kT = persist.tile([128, L], BF16, tag="kT", name="kT")  # [d, l]
    vN = persist.tile([128, L], BF16, tag="vN", name="vN")  # block j: [:, 128j:+128] = V[128j:+128, :]
    w_ch = [persist.tile([128, WQK], BF16, tag=f"w{k}", name=f"w{k}") for k in range(DC)]
    ones = persist.tile([128, 128], BF16, tag="ones", name="ones")
    ident = persist.tile([128, 128], BF16, tag="ident", name="ident")
    bq_sb = persist.tile([128, NH], F32, tag="bq", name="bq")
    bqr_sb = persist.tile([128, NH * 512], F32, tag="bqr", name="bqr")
    bk_sb = persist.tile([128, 1], F32, tag="bk", name="bk")
    bv_sb = persist.tile([128, 1], F32, tag="bv", name="bv")

    def wq(k, h):
        return w_ch[k][:, h * 128:(h + 1) * 128]

    nc.vector.memset(ones[:], 1.0)
    nc.vector.memset(bqr_sb[:], 0.0)
    masks.make_identity(nc, ident[:])
    nc.sync.dma_start(out=bq_sb[:], in_=bq)
    nc.sync.dma_start(out=bk_sb[:], in_=bk)
    nc.sync.dma_start(out=bv_sb[:], in_=bv)
    # per-head Q bias replicated along l (for DVE-side evacuation adds):
    # out = 0*0 + bias via ACT, no DMA needed
    for h in range(NH):
        nc.scalar.activation(
            bqr_sb[:, h * 512:(h + 1) * 512], bqr_sb[:, h * 512:(h + 1) * 512],
            AF.Identity, scale=0.0, bias=bq_sb[:, h:h + 1],
        )

    # Big-line input DMAs alternating between the two HW DGE queues.
    dq = [nc.sync, nc.scalar]
    for k in range(DC):
        rs = slice(k * 128, (k + 1) * 128)
        dq[k % 2].dma_start(out=w_ch[k][:], in_=wall[rs, :])
        dq[(k + 1) % 2].dma_start(out=x_sb[k][:], in_=xT[rs, :])

    # ---------------- Phase 1: K/V projections (all L) + Q for lq=0 ----------
    with (
        tc.tile_pool(name="pjkv", bufs=2, space="PSUM") as pjkv,
        tc.tile_pool(name="q0p", bufs=1, space="PSUM") as q0p,
        tc.tile_pool(name="tp", bufs=2, space="PSUM") as tpp,
        tc.tile_pool(name="vt", bufs=2) as vtp,
    ):
        for lt in range(NLT):
            ls = slice(lt * 512, (lt + 1) * 512)
            psk = pjkv.tile([128, 512], F32, tag="a", name=f"psk{lt}")
            psv = pjkv.tile([128, 512], F32, tag="b", name=f"psv{lt}")
            # Q0 half-pass (2 heads) folded into l-tiles 2 and 3
            qh = None
            if lt >= 2:
                h0, h1 = 2 * (lt - 2), 2 * (lt - 2) + 1
                pa = q0p.tile([128, 512], F32, tag="qa", name=f"q0a{lt}")
                pb = q0p.tile([128, 512], F32, tag="qb", name=f"q0b{lt}")
                qh = (h0, h1, pa, pb)
            for k in range(DC):
                st, sp = k == 0, k == DC - 1
                nc.tensor.matmul(psk[:], lhsT=w_ch[k][:, QC:QC + 128], rhs=x_sb[k][:, ls], start=st, stop=sp)
                nc.tensor.matmul(psv[:], lhsT=w_ch[k][:, QC + 128:QC + 256], rhs=x_sb[k][:, ls], start=st, stop=sp)
                if qh is not None:
                    h0, h1, pa, pb = qh
                    nc.tensor.matmul(pa[:], lhsT=wq(k, h0), rhs=x_sb[k][:, 0:512], start=st, stop=sp)
                    nc.tensor.matmul(pb[:], lhsT=wq(k, h1), rhs=x_sb[k][:, 0:512], start=st, stop=sp)
            nc.scalar.activation(kT[:, ls], psk[:], AF.Identity, bias=bk_sb[:, 0:1])
            vt = vtp.tile([128, 512], BF16, tag="vt", name=f"vt{lt}")
            nc.scalar.activation(vt[:], psv[:], AF.Identity, bias=bv_sb[:, 0:1])
            if qh is not None:
                h0, h1, pa, pb = qh
                nc.scalar.activation(qT[h0][:, 0:512], pa[:], AF.Identity, bias=bq_sb[:, h0:h0 + 1])
                nc.scalar.activation(qT[h1][:, 0:512], pb[:], AF.Identity, bias=bq_sb[:, h1:h1 + 1])
            # transpose this l-tile of V to natural layout right away
            for jj in range(4):
                j = lt * 4 + jj
                pt = tpp.tile([128, 128], BF16, tag="tp", name=f"tp{j}")
                nc.tensor.transpose(pt[:], vt[:, jj * 128:(jj + 1) * 128], ident[:])
                nc.scalar.activation(vN[:, j * 128:(j + 1) * 128], pt[:], AF.Identity)

    # ---------------- Phase 2: attention + interleaved Q-proj ----------------
    with (
        tc.tile_pool(name="pj2", bufs=1, space="PSUM") as pj2,  # 1 bank
        tc.tile_pool(name="ssp", bufs=2, space="PSUM") as ssp,  # 4 banks
        tc.tile_pool(name="avp", bufs=1, space="PSUM") as avp,  # 2 banks
        tc.tile_pool(name="rrp", bufs=1, space="PSUM") as rrp,  # 1 bank
        tc.tile_pool(name="att", bufs=10) as attp,
        tc.tile_pool(name="acc", bufs=2) as accp,
        tc.tile_pool(name="fin", bufs=2) as finp,
    ):
        def emit_av(p):
            at, psA, ks, st, sp = p
            for j in range(2):
                nc.tensor.matmul(
                    psA[j][:], lhsT=vN[:, ks], rhs=at[:, j * 512:(j + 1) * 512],
                    start=st, stop=sp,
                )

        def make_qproj(lq):
            """32 steps x (2 matmuls of one k-chunk-pair) of the Q projection
            for q-block lq+1, as 4 single-head quarter passes (1 PSUM bank)."""
            if lq >= NLQ - 1:
                while True:
                    yield
            nls = slice((lq + 1) * 512, (lq + 2) * 512)
            for h in range(NH):
                pq = pj2.tile([128, 512], F32, tag="q", name=f"pq{lq}{h}")
                for s in range(8):
                    for k in (2 * s, 2 * s + 1):
                        nc.tensor.matmul(pq[:], lhsT=wq(k, h), rhs=x_sb[k][:, nls], start=k == 0, stop=k == DC - 1)
                    if s == 7:
                        nc.vector.tensor_add(qT[h][:, nls], pq[:], bqr_sb[:, h * 512:(h + 1) * 512])
                    yield
            while True:
                yield

        def make_finalize(j, h, qs, c01, c23, otu, tail):
            def fi():
                # r for this head: reduce the combined partial sums over key
                # partitions (two accumulating ones-matmuls, replicated out)
                rr = rrp.tile([128, 512], F32, tag="rr", name=f"rr{h}{qs.start}")
                nc.tensor.matmul(rr[:], lhsT=ones[:], rhs=c01[:, j * 512:(j + 1) * 512], start=True, stop=False)
                nc.tensor.matmul(rr[:], lhsT=ones[:], rhs=c23[:, j * 512:(j + 1) * 512], start=False, stop=True)
                rrs = finp.tile([128, 512], F32, tag="rrs", name=f"rrs{h}{qs.start}")
                nc.vector.tensor_copy(rrs[:], rr[:])
                rinv = finp.tile([128, 512], F32, tag="rinv", name=f"ri{h}{qs.start}")
                if j == 1 or tail:
                    # ACT-side reciprocal: exp(-ln r); Ln and Exp share the
                    # natural_log_exp_and_others table set -> no table thrash
                    lnr = finp.tile([128, 512], F32, tag="lnr", name=f"ln{h}{qs.start}")
                    nc.scalar.activation(lnr[:], rrs[:], AF.Ln)
                    nc.scalar.activation(rinv[:], lnr[:], AF.Exp, scale=-1.0)
                else:
                    nc.vector.reciprocal(rinv[:], rrs[:])
                ot = finp.tile([128, 512], F32, tag="ot", name=f"o{h}{qs.start}")
                nc.vector.tensor_mul(ot[:], otu[:], rinv[:])
                nc.sync.dma_start(out=outT[h * 128:(h + 1) * 128, qs], in_=ot[:])
            return fi

        deferred = []
        for lq in range(NLQ):
            qs = slice(lq * 512, (lq + 1) * 512)
            qsteps = make_qproj(lq)
            for hp in range(2):
                psA = [avp.tile([128, 512], F32, tag=f"av{j}", name=f"av{lq}{hp}{j}") for j in range(2)]
                pend = []
                chains = [None] * 4
                for lk in range(LKT):
                    ks = slice(lk * 128, (lk + 1) * 128)
                    ss = ssp.tile([128, 1024], F32, tag="ss", name=f"ss{lq}{hp}{lk}")
                    for j in range(2):
                        nc.tensor.matmul(
                            ss[:, j * 512:(j + 1) * 512],
                            lhsT=kT[:, ks], rhs=qT[2 * hp + j][:, qs],
                            start=True, stop=True,
                        )
                    at = attp.tile([128, 1024], BF16, tag="at", name=f"at{lq}{hp}{lk}")
                    nc.scalar.activation(at[:], ss[:], AF.Exp, scale=SCALE)
                    # 4 interleaved partial-sum chains; chains < GP_CHAINS on GpSimd
                    c = lk % 4
                    if lk < 4:
                        chains[c] = at
                    else:
                        a_new = accp.tile([128, 1024], BF16, tag=f"ch{c}", name=f"ch{lq}{hp}{lk}")
                        eng = nc.gpsimd if c < GP_CHAINS else nc.vector
                        eng.tensor_add(a_new[:], chains[c][:], at[:])
                        chains[c] = a_new
                    # software-pipelined AV (consume an older chunk's exp)
                    pend.append((at, psA, ks, lk == 0, lk == LKT - 1))
                    if len(pend) > 4:
                        emit_av(pend.pop(0))
                    # previous pass's finals, mid-pass so nothing waits
                    if lk == 4 and deferred:
                        deferred.pop(0)()
                    if lk == 8 and deferred:
                        deferred.pop(0)()
                    next(qsteps)
                while pend:
                    emit_av(pend.pop(0))
                # pairwise chain combines (DVE; GpSimd chains land in c01),
                # evacuate AV PSUM to SBUF right away so the AV banks and all
                # downstream finals are decoupled from the next pass
                c01 = accp.tile([128, 1024], BF16, tag="cA", name=f"cA{lq}{hp}")
                nc.vector.tensor_add(c01[:], chains[0][:], chains[1][:])
                c23 = accp.tile([128, 1024], BF16, tag="cB", name=f"cB{lq}{hp}")
                nc.vector.tensor_add(c23[:], chains[2][:], chains[3][:])
                tail = lq == NLQ - 1 and hp == 1
                for j in range(2):
                    otu = finp.tile([128, 512], F32, tag=f"otu{j}", name=f"otu{lq}{hp}{j}")
                    nc.vector.tensor_copy(otu[:], psA[j][:])
                    deferred.append(make_finalize(j, 2 * hp + j, qs, c01, c23, otu, tail))
        while deferred:
            deferred.pop(0)()


_NC_CACHE = None


def build_nc():
    global _NC_CACHE
    if _NC_CACHE is not None:
        return _NC_CACHE
    nc = bacc.Bacc("TRN2", target_bir_lowering=False, debug=False)
    xT = nc.dram_tensor("xT", [D, L], BF16, kind="ExternalInput").ap()
    wall = nc.dram_tensor("wall", [D, WQK], BF16, kind="ExternalInput").ap()
    bq = nc.dram_tensor("bq", [128, NH], F32, kind="ExternalInput").ap()
    bk = nc.dram_tensor("bk", [128, 1], F32, kind="ExternalInput").ap()
    bv = nc.dram_tensor("bv", [128, 1], F32, kind="ExternalInput").ap()
    outT = nc.dram_tensor("outT", [QC, L], F32, kind="ExternalOutput").ap()
    with tile.TileContext(nc) as tc, ExitStack() as ctx:
        build_kernel(ctx, tc, xT, wall, bq, bk, bv, outT)
    nc.compile()
    _NC_CACHE = nc
    return nc


def make_in_maps(x, Wq_w, Wq_b, Wk_w, Wk_b, Wv_w, Wv_b):
    """Host-side sharding/relayout. Returns one input map per core."""
    bf16 = ml_dtypes.bfloat16
    x = np.asarray(x, dtype=np.float32)
    Wq_w = np.asarray(Wq_w, dtype=np.float32)
    Wq_b = np.asarray(Wq_b, dtype=np.float32)
    Wk_w = np.asarray(Wk_w, dtype=np.float32)
    Wk_b = np.asarray(Wk_b, dtype=np.float32)
    Wv_w = np.asarray(Wv_w, dtype=np.float32)
    Wv_b = np.asarray(Wv_b, dtype=np.float32)

    xTs = [np.ascontiguousarray(x[b].T).astype(bf16) for b in range(B)]
    wkT = Wk_w.T
    wvT = Wv_w.T
    bk = np.ascontiguousarray(Wk_b.reshape(128, 1))
    bv = np.ascontiguousarray(Wv_b.reshape(128, 1))
    in_maps = []
    for c in range(N_CORES):
        b, g = divmod(c, B * 2)  # b = c // 4, g = c % 4
        wqT_g = Wq_w[g * QC:(g + 1) * QC, :].T
        wall = np.ascontiguousarray(
            np.concatenate([wqT_g, wkT, wvT], axis=1)
        ).astype(bf16)
        bq_g = np.ascontiguousarray(Wq_b[g * QC:(g + 1) * QC].reshape(NH, 128).T)
        in_maps.append(
            {
                "xT": xTs[b],
                "wall": wall,
                "bq": bq_g,
                "bk": bk,
                "bv": bv,
            }
        )
    return in_maps


def assemble_output(results):
    out = np.empty((B, L, D), dtype=np.float32)
    for c in range(N_CORES):
        b, g = divmod(c, B * 2)
        out[b, :, g * QC:(g + 1) * QC] = results[c]["outT"].T
    return out


def kernel(**inputs) -> np.ndarray:
    nc = build_nc()
    in_maps = make_in_maps(**inputs)
    res = run_bass_kernel_spmd(nc, in_maps, core_ids=list(range(N_CORES)))
    return assemble_output(res.results)


# revision 10
# speedup vs baseline: 1.1862x; 1.0670x over previous
"""Multi-Query Attention kernel for 8x TRN2 NeuronCores (Bass/Tile).

Problem: x[B=2, L=2048, D=2048], Wq[2048,2048], Wk/Wv[128,2048] (MQA: one
shared K/V head), 16 query heads of dim 128.

Sharding: core c in [0,8): batch b = c//4, head-group g = c%4 (4 heads,
i.e. q-channels [512g, 512g+512)). K/V replicated per core.

V5 design (all-bf16 matmul path, phase-fused, stall-free pass boundaries):
  - x resident in SBUF, DMA'd as whole [128,2048] chunks (4KB partition
    lines; small lines ran at ~half DMA rate and paced all of phase 1),
    transfers alternating between the SP and Activation HW DGE queues;
    Wq/Wk/Wv are concatenated host-side into one [D,768] tensor so weight
    chunks also move with 1.5KB lines
  - all matmuls bf16 (same PE rate as f32r, but FWL + LDWEIGHTS pull-ahead
    hide weight loads). fp8/DoubleRow rejected: ~2.4% RMS quantization
    noise propagates undamped through dots -> breaches the 2e-2 gate.
  - phase 1: K/V projections (+ PE transposes of V to natural layout);
    Q projection for q-block 0 folded into l-tiles 2/3
  - phase 2: per q-block: attention passes with the NEXT q-block's Q
    projection interleaved one 2-matmul step per iteration (PE stays busy
    while ACT chews the 1.15us-per-tile exp stream)
  - softmax denominator: 4 interleaved partial-sum chains over the exp'd
    tiles (2 on GpSimd, 2 on DVE; fully serial chains pay a read-write
    bubble per op, and GpSimd contends with DVE for the shared SBUF port,
    so the load is split), pairwise-combined on DVE; the cross-partition
    reduction is two accumulating ones-matmuls per head into a dedicated
    1-bank PSUM slot
  - pass finals are DEFERRED into the middle of the NEXT pass and fully
    decoupled from PSUM (AV accumulators and r are evacuated to SBUF
    immediately), so no PSUM rotation and no PE instruction ever waits on
    the multi-microsecond reciprocal; head 0 inverts on DVE, head 1 on
    ACT via exp(-ln(r)) (both functions live in one table set)
  - PSUM: Qproj 1 + scores 2x2 + AV 2 + r 1 = 8 banks exactly
"""

from contextlib import ExitStack

import numpy as np
import ml_dtypes

import concourse.bass as bass
import concourse.tile as tile
from concourse import bacc, masks, mybir
from concourse.bass_utils import run_bass_kernel_spmd

F32 = mybir.dt.float32
BF16 = mybir.dt.bfloat16
AF = mybir.ActivationFunctionType

B = 2
L = 2048
D = 2048  # d_model (contraction dim of projections)
HD = 128  # head dim
NH = 4  # heads per core
QC = NH * HD  # q-channels per core = 512
DC = D // 128  # d-model chunks of 128 = 16
NLT = 4  # l tiles of 512 (projection)
LKT = L // 128  # lk blocks of 128 = 16
NLQ = 4  # lq blocks of 512 (attention)
WQK = QC + 2 * HD  # combined weight row: 512 wq | 128 wk | 128 wv
N_CORES = 8
SCALE = 1.0 / float(np.sqrt(HD))

GP_CHAINS = 1  # how many of the 4 partial-sum chains run on GpSimd


def build_kernel(ctx: ExitStack, tc: tile.TileContext, xT, wall, bq, bk, bv, outT):
    nc = tc.nc

    persist = ctx.enter_context(tc.tile_pool(name="persist", bufs=1))
    x_sb = [persist.tile([128, L], BF16, tag=f"x{k}", name=f"x{k}") for k in range(DC)]
    qT = [persist.tile([128, L], BF16, tag=f"qT{h}", name=f"qT{h}") for h in range(NH)]  # [d, l]
    kT = persist.tile([128, L], BF16, tag="kT", name="kT")  # [d, l]
    vN = persist.tile([128, L], BF16, tag="vN", name="vN")  # block j: [:, 128j:+128] = V[128j:+128, :]
    w_ch = [persist.tile([128, WQK], BF16, tag=f"w{k}", name=f"w{k}") for k in range(DC)]
    ones = persist.tile([128, 128], BF16, tag="ones", name="ones")
    ident = persist.tile([128, 128], BF16, tag="ident", name="ident")
    bq_sb = persist.tile([128, NH], F32, tag="bq", name="bq")
    bqr_sb = persist.tile([128, NH * 512], F32, tag="bqr", name="bqr")
    bk_sb = persist.tile([128, 1], F32, tag="bk", name="bk")
    bv_sb = persist.tile([128, 1], F32, tag="bv", name="bv")

    def wq(k, h):
        return w_ch[k][:, h * 128:(h + 1) * 128]

    nc.vector.memset(ones[:], 1.0)
    nc.vector.memset(bqr_sb[:], 0.0)
    masks.make_identity(nc, ident[:])
    nc.sync.dma_start(out=bq_sb[:], in_=bq)
    nc.sync.dma_start(out=bk_sb[:], in_=bk)
    nc.sync.dma_start(out=bv_sb[:], in_=bv)
    # per-head Q bias replicated along l (for DVE-side evacuation adds):
    # out = 0*0 + bias via ACT, no DMA needed
    for h in range(NH):
        nc.scalar.activation(
            bqr_sb[:, h * 512:(h + 1) * 512], bqr_sb[:, h * 512:(h + 1) * 512],
            AF.Identity, scale=0.0, bias=bq_sb[:, h:h + 1],
        )

    # Big-line input DMAs alternating between the two HW DGE queues.
    dq = [nc.sync, nc.scalar]
    for k in range(DC):
        rs = slice(k * 128, (k + 1) * 128)
        dq[k % 2].dma_start(out=w_ch[k][:], in_=wall[rs, :])
        dq[(k + 1) % 2].dma_start(out=x_sb[k][:, 0:1024], in_=xT[rs, 0:1024])
    for k in range(DC):
        rs = slice(k * 128, (k + 1) * 128)
        dq[k % 2].dma_start(out=x_sb[k][:, 1024:2048], in_=xT[rs, 1024:2048])

    # ---------------- Phase 1: K/V projections (all L) + Q for lq=0 ----------
    with (
        tc.tile_pool(name="pjkv", bufs=2, space="PSUM") as pjkv,
        tc.tile_pool(name="q0p", bufs=1, space="PSUM") as q0p,
        tc.tile_pool(name="tp", bufs=2, space="PSUM") as tpp,
        tc.tile_pool(name="vt", bufs=2) as vtp,
    ):
        for lt in range(NLT):
            ls = slice(lt * 512, (lt + 1) * 512)
            psk = pjkv.tile([128, 512], F32, tag="a", name=f"psk{lt}")
            psv = pjkv.tile([128, 512], F32, tag="b", name=f"psv{lt}")
            # Q0 half-pass (2 heads) folded into l-tiles 2 and 3
            qh = None
            if lt >= 2:
                h0, h1 = 2 * (lt - 2), 2 * (lt - 2) + 1
                pa = q0p.tile([128, 512], F32, tag="qa", name=f"q0a{lt}")
                pb = q0p.tile([128, 512], F32, tag="qb", name=f"q0b{lt}")
                qh = (h0, h1, pa, pb)
            for k in range(DC):
                st, sp = k == 0, k == DC - 1
                nc.tensor.matmul(psk[:], lhsT=w_ch[k][:, QC:QC + 128], rhs=x_sb[k][:, ls], start=st, stop=sp)
                nc.tensor.matmul(psv[:], lhsT=w_ch[k][:, QC + 128:QC + 256], rhs=x_sb[k][:, ls], start=st, stop=sp)
                if qh is not None:
                    h0, h1, pa, pb = qh
                    nc.tensor.matmul(pa[:], lhsT=wq(k, h0), rhs=x_sb[k][:, 0:512], start=st, stop=sp)
                    nc.tensor.matmul(pb[:], lhsT=wq(k, h1), rhs=x_sb[k][:, 0:512], start=st, stop=sp)
            nc.scalar.activation(kT[:, ls], psk[:], AF.Identity, bias=bk_sb[:, 0:1])
            vt = vtp.tile([128, 512], BF16, tag="vt", name=f"vt{lt}")
            nc.scalar.activation(vt[:], psv[:], AF.Identity, bias=bv_sb[:, 0:1])
            if qh is not None:
                h0, h1, pa, pb = qh
                nc.scalar.activation(qT[h0][:, 0:512], pa[:], AF.Identity, bias=bq_sb[:, h0:h0 + 1])
                nc.scalar.activation(qT[h1][:, 0:512], pb[:], AF.Identity, bias=bq_sb[:, h1:h1 + 1])
            # transpose this l-tile of V to natural layout right away
            for jj in range(4):
                j = lt * 4 + jj
                pt = tpp.tile([128, 128], BF16, tag="tp", name=f"tp{j}")
                nc.tensor.transpose(pt[:], vt[:, jj * 128:(jj + 1) * 128], ident[:])
                nc.scalar.activation(vN[:, j * 128:(j + 1) * 128], pt[:], AF.Identity)

    # ---------------- Phase 2: attention + interleaved Q-proj ----------------
    with (
        tc.tile_pool(name="pj2", bufs=1, space="PSUM") as pj2,  # 1 bank
        tc.tile_pool(name="ssp", bufs=2, space="PSUM") as ssp,  # 4 banks
        tc.tile_pool(name="avp", bufs=1, space="PSUM") as avp,  # 2 banks
        tc.tile_pool(name="rrp", bufs=1, space="PSUM") as rrp,  # 1 bank
        tc.tile_pool(name="att", bufs=10) as attp,
        tc.tile_pool(name="acc", bufs=3) as accp,
        tc.tile_pool(name="fin", bufs=2) as finp,
    ):
        def emit_av(p):
            at, psA, ks, st, sp = p
            for j in range(2):
                nc.tensor.matmul(
                    psA[j][:], lhsT=vN[:, ks], rhs=at[:, j * 512:(j + 1) * 512],
                    start=st, stop=sp,
                )

        def make_qproj(lq):
            """32 steps x (2 matmuls of one k-chunk-pair) of the Q projection
            for q-block lq+1, as 4 single-head quarter passes (1 PSUM bank)."""
            if lq >= NLQ - 1:
                while True:
                    yield
            nls = slice((lq + 1) * 512, (lq + 2) * 512)
            for h in range(NH):
                pq = pj2.tile([128, 512], F32, tag="q", name=f"pq{lq}{h}")
                for s in range(8):
                    for k in (2 * s, 2 * s + 1):
                        nc.tensor.matmul(pq[:], lhsT=wq(k, h), rhs=x_sb[k][:, nls], start=k == 0, stop=k == DC - 1)
                    if s == 7:
                        nc.vector.tensor_add(qT[h][:, nls], pq[:], bqr_sb[:, h * 512:(h + 1) * 512])
                    yield
            while True:
                yield

        def make_finalize(j, h, qs, chs, otu, tail):
            def fi():
                # r for this head: reduce the partial-sum chains over key
                # partitions - four accumulating ones-matmuls, one per chain,
                # so each depends only on its own chain's last add (no DVE
                # combine on the critical path: the Tile scheduler hoists
                # these matmuls, and in V5 they stalled the PE every pass
                # waiting on a combine that waited on GpSimd)
                rr = rrp.tile([128, 512], F32, tag="rr", name=f"rr{h}{qs.start}")
                for c in range(4):
                    nc.tensor.matmul(rr[:], lhsT=ones[:], rhs=chs[c][:, j * 512:(j + 1) * 512], start=c == 0, stop=c == 3)
                rinv = finp.tile([128, 512], F32, tag="rinv", name=f"ri{h}{qs.start}")
                if j == 1 or tail:
                    # ACT-side reciprocal: exp(-ln r); Ln and Exp share the
                    # natural_log_exp_and_others table set -> no table thrash
                    lnr = finp.tile([128, 512], F32, tag="lnr", name=f"ln{h}{qs.start}")
                    nc.scalar.activation(lnr[:], rr[:], AF.Ln)
                    nc.scalar.activation(rinv[:], lnr[:], AF.Exp, scale=-1.0)
                else:
                    nc.vector.reciprocal(rinv[:], rr[:])
                ot = finp.tile([128, 512], F32, tag="ot", name=f"o{h}{qs.start}")
                nc.vector.tensor_mul(ot[:], otu[:], rinv[:])
                nc.sync.dma_start(out=outT[h * 128:(h + 1) * 128, qs], in_=ot[:])
            return fi

        deferred = []
        for lq in range(NLQ):
            qs = slice(lq * 512, (lq + 1) * 512)
            qsteps = make_qproj(lq)
            for hp in range(2):
                psA = [avp.tile([128, 512], F32, tag=f"av{j}", name=f"av{lq}{hp}{j}") for j in range(2)]
                pend = []
                chains = [None] * 4
                for lk in range(LKT):
                    ks = slice(lk * 128, (lk + 1) * 128)
                    ss = ssp.tile([128, 1024], F32, tag="ss", name=f"ss{lq}{hp}{lk}")
                    for j in range(2):
                        nc.tensor.matmul(
                            ss[:, j * 512:(j + 1) * 512],
                            lhsT=kT[:, ks], rhs=qT[2 * hp + j][:, qs],
                            start=True, stop=True,
                        )
                    at = attp.tile([128, 1024], BF16, tag="at", name=f"at{lq}{hp}{lk}")
                    nc.scalar.activation(at[:], ss[:], AF.Exp, scale=SCALE)
                    # 4 interleaved partial-sum chains; chains < GP_CHAINS on GpSimd
                    c = lk % 4
                    if lk < 4:
                        chains[c] = at
                    else:
                        a_new = accp.tile([128, 1024], BF16, tag=f"ch{c}", name=f"ch{lq}{hp}{lk}")
                        eng = nc.gpsimd if c < GP_CHAINS else nc.vector
                        eng.tensor_add(a_new[:], chains[c][:], at[:])
                        chains[c] = a_new
                    # software-pipelined AV (consume an older chunk's exp)
                    pend.append((at, psA, ks, lk == 0, lk == LKT - 1))
                    if len(pend) > 4:
                        emit_av(pend.pop(0))
                    # previous pass's finals, mid-pass so nothing waits
                    if lk == 8 and deferred:
                        deferred.pop(0)()
                    if lk == 11 and deferred:
                        deferred.pop(0)()
                    next(qsteps)
                while pend:
                    emit_av(pend.pop(0))
                # evacuate AV PSUM to SBUF right away so the AV banks and all
                # downstream finals are decoupled from the next pass
                tail = lq == NLQ - 1 and hp == 1
                chs = tuple(chains)
                for j in range(2):
                    otu = finp.tile([128, 512], F32, tag=f"otu{j}", name=f"otu{lq}{hp}{j}")
                    nc.vector.tensor_copy(otu[:], psA[j][:])
                    deferred.append(make_finalize(j, 2 * hp + j, qs, chs, otu, tail))
        while deferred:
            deferred.pop(0)()


_NC_CACHE = None


def build_nc():
    global _NC_CACHE
    if _NC_CACHE is not None:
        return _NC_CACHE
    nc = bacc.Bacc("TRN2", target_bir_lowering=False, debug=False)
    xT = nc.dram_tensor("xT", [D, L], BF16, kind="ExternalInput").ap()
    wall = nc.dram_tensor("wall", [D, WQK], BF16, kind="ExternalInput").ap()
    bq = nc.dram_tensor("bq", [128, NH], F32, kind="ExternalInput").ap()
    bk = nc.dram_tensor("bk", [128, 1], F32, kind="ExternalInput").ap()
    bv = nc.dram_tensor("bv", [128, 1], F32, kind="ExternalInput").ap()
    outT = nc.dram_tensor("outT", [QC, L], F32, kind="ExternalOutput").ap()
    with tile.TileContext(nc) as tc, ExitStack() as ctx:
        build_kernel(ctx, tc, xT, wall, bq, bk, bv, outT)
    nc.compile()
    _NC_CACHE = nc
    return nc


def make_in_maps(x, Wq_w, Wq_b, Wk_w, Wk_b, Wv_w, Wv_b):
    """Host-side sharding/relayout. Returns one input map per core."""
    bf16 = ml_dtypes.bfloat16
    x = np.asarray(x, dtype=np.float32)
    Wq_w = np.asarray(Wq_w, dtype=np.float32)
    Wq_b = np.asarray(Wq_b, dtype=np.float32)
    Wk_w = np.asarray(Wk_w, dtype=np.float32)
    Wk_b = np.asarray(Wk_b, dtype=np.float32)
    Wv_w = np.asarray(Wv_w, dtype=np.float32)
    Wv_b = np.asarray(Wv_b, dtype=np.float32)

    xTs = [np.ascontiguousarray(x[b].T).astype(bf16) for b in range(B)]
    wkT = Wk_w.T
    wvT = Wv_w.T
    bk = np.ascontiguousarray(Wk_b.reshape(128, 1))
    bv = np.ascontiguousarray(Wv_b.reshape(128, 1))
    in_maps = []
    for c in range(N_CORES):
        b, g = divmod(c, B * 2)  # b = c // 4, g = c % 4
        wqT_g = Wq_w[g * QC:(g + 1) * QC, :].T
        wall = np.ascontiguousarray(
            np.concatenate([wqT_g, wkT, wvT], axis=1)
        ).astype(bf16)
        bq_g = np.ascontiguousarray(Wq_b[g * QC:(g + 1) * QC].reshape(NH, 128).T)
        in_maps.append(
            {
                "xT": xTs[b],
                "wall": wall,
                "bq": bq_g,
                "bk": bk,
                "bv": bv,
            }
        )
    return in_maps


def assemble_output(results):
    out = np.empty((B, L, D), dtype=np.float32)
    for c in range(N_CORES):
        b, g = divmod(c, B * 2)
        out[b, :, g * QC:(g + 1) * QC] = results[c]["outT"].T
    return out


def kernel(**inputs) -> np.ndarray:
    nc = build_nc()
    in_maps = make_in_maps(**inputs)
    res = run_bass_kernel_spmd(nc, in_maps, core_ids=list(range(N_CORES)))
    return assemble_output(res.results)
